# revision 3
# baseline (speedup 1.0000x reference)
"""Trainium2 Bass kernel for nn_CustomTransformer_60619168416497.

kernel(**inputs) takes the FULL unsharded inputs (as produced by
setup_inputs()) and returns the FULL output (scalar f32 loss), running the
heavy X-dependent work on 8 NeuronCores (data parallel over the batch).

-- Algebraic reduction -------------------------------------------------------
Only h_2[:, -1] (the cls row) reaches the output head, so the full attention
never needs to be materialized. Folding the tiny weight matrices on the host:
    w    = W1 @ W_k @ (cls@W_q) / sqrt(32)        [8]
    N    = W1 @ W_v @ W2                          [8,2]
    a_cls= cls . (W_k @ (cls@W_q))/sqrt(32)       scalar
per batch b with normalized x = (X - mu)/sigma':
    token logit l_j = alpha*(t_j - mu*sum(w)),  t_j = X[b,j,:]@w
    cls logit      = a_cls
    S = softmax over the 257 logits; only two functionals of X are needed:
      denom-part  sum_j e_j   and   G2 = sum_j e_j * (X[b,j,:]@N)
    from which z[b] and the NLL follow in closed form (host, f64).
-- Device work (per core, 256 batches) --------------------------------------
Launch 1: global sum / sumsq partials of X  ->  host computes mu, sigma.
Launch 2: per batch M_t = max_j t_j, e = exp(alpha*(t - M_t)),
          denom = sum e, G2 = sum e*r  ->  host finishes the loss.
Layout: "batch-partition planes" A[i][p][col] (col = bh*256 + j, local batch
  = bh*128 + p) with A_i = bf16(w_i * X_i) pre-scaled on the host. Per-token
  contractions over i become 8 PSUM-accumulating matmuls with identity /
  diagonal stationary weights (PE streams 1 column/cycle); softmax pieces run
  on ACT (exp with fused scale/bias/accum) and DVE (max, products, sums).
Both launches read the same 1.05 MB/core of planes. The two NEFFs are
input-independent (all data arrives via input tensors), so compilation is
cacheable across calls and inputs.
"""
import numpy as np
import ml_dtypes

import concourse.tile as tile
import concourse.mybir as mybir
from concourse import bacc
from concourse.bass_utils import run_bass_kernel_spmd

F32 = mybir.dt.float32
BF16 = mybir.dt.bfloat16
NCORES = 8
BPC = 256          # batches per core
L = 256            # tokens
I = 8              # features
COLS = 512         # bh*256 + j
H = 32
EPS = 1e-7
STATS_NCHUNK = 8
MAIN_NCHUNK = 4
# NOTE on op choices: tensor_tensor_reduce is a custom DVE op that the
# PJRT/axon runtime cannot execute (crashes the exec unit), so G2 uses plain
# tensor_mul + tensor_reduce. ACT Exp carries fused scale/bias APs and
# accum_out; bn_stats carries both sum and sumsq per plane in one pass.

bf16 = ml_dtypes.bfloat16


# ---------------------------------------------------------------- host math
def _fold_weights(W1, cls_tok, W_q, W_k, W_v, W_t, W2):
    f8 = np.float64
    W1, cls_tok, W_q, W_k, W_v, W_t, W2 = [np.asarray(a, f8) for a in
                                           (W1, cls_tok, W_q, W_k, W_v, W_t, W2)]
    Q = cls_tok @ W_q
    u = (W_k @ Q) / np.sqrt(f8(H))
    w = W1 @ u
    N = (W1 @ W_v) @ W2
    return dict(
        w=w, N=N,
        a_cls=float(cls_tok @ u),
        sumw=float(w.sum()),
        n1=N.sum(axis=0),
        v2=(cls_tok @ W_v) @ W2,
        t2=(cls_tok @ W_t) @ W2,
    )


def _prep_inputs(X, w):
    X = np.ascontiguousarray(np.asarray(X, np.float32))
    w32 = np.asarray(w, np.float32)
    if np.abs(w32).min() < 1e-3 * max(np.abs(w32).max(), 1.0):
        raise RuntimeError("w has near-zero entries; scaled-plane trick unsafe")
    A = (X * w32[None, None, :]).astype(bf16)
    per_core = []
    for c in range(NCORES):
        a = A[c * BPC:(c + 1) * BPC].reshape(2, 128, L, I)   # [bh, p, j, i]
        per_core.append(
            np.ascontiguousarray(a.transpose(3, 1, 0, 2)).reshape(I, 128, COLS))
    return per_core


def _build_aux(fold, alpha):
    aux = np.zeros((128, 18), np.float32)
    aux[:, 0] = alpha
    aux[:, 1] = -alpha
    coef = (fold["N"] / fold["w"][:, None]).astype(np.float32)
    aux[:, 2:10] = coef[:, 0][None, :]
    aux[:, 10:18] = coef[:, 1][None, :]
    return aux


def _chunked_plane_dma(nc, pool, src_dram, tag, nchunk):
    pp = I // nchunk
    src = src_dram.rearrange("i p c -> p i c")
    lookup = {}
    for ch in range(nchunk):
        t = pool.tile([128, pp * COLS], BF16, tag=f"{tag}{ch}",
                      name=f"{tag}{ch}")
        dst = t[:].rearrange("p (i c) -> p i c", i=pp)
        eng = nc.sync if ch % 2 == 0 else nc.scalar
        eng.dma_start(dst[:, :, :], src[:, ch * pp:(ch + 1) * pp, :])
        for k in range(pp):
            lookup[ch * pp + k] = (t, k * COLS)
    return lookup


# ---------------------------------------------------------------- kernel 1
def _stats_body(nc):
    """All 8 planes via DVE bn_stats -> sc [128, 48] (6 cols per plane:
    count, mean, M2 for even and odd element halves)."""
    sp = nc.dram_tensor("sp", [I, 128, COLS], BF16, kind="ExternalInput")
    sc = nc.dram_tensor("sc", [128, 48], F32, kind="ExternalOutput")
    with tile.TileContext(nc) as tc:
        with (
            tc.tile_pool(name="xpool", bufs=1) as xpool,
            tc.tile_pool(name="outp", bufs=1) as outp,
        ):
            out = outp.tile([128, 48], F32, name="out", tag="out")
            planes = _chunked_plane_dma(nc, xpool, sp, "x", STATS_NCHUNK)
            for i in range(I):
                t, c0 = planes[i]
                nc.vector.bn_stats(out[:, 6 * i:6 * i + 6], t[:, c0:c0 + COLS])
            nc.sync.dma_start(sc[:], out[:])
    return nc


def _host_stats(res_list, w):
    w = np.asarray(w, np.float64)
    s1 = s2 = 0.0
    for r in res_list:
        sc = np.asarray(r["sc"]).astype(np.float64)
        bn = sc.reshape(128, I, 2, 3)
        cnt, mean, m2 = bn[..., 0], bn[..., 1], bn[..., 2]
        s1 += ((cnt * mean).sum(axis=(0, 2)) / w).sum()
        s2 += ((m2 + cnt * mean * mean).sum(axis=(0, 2)) / w ** 2).sum()
    n = NCORES * BPC * L * I
    mu = s1 / n
    var = (s2 - n * mu * mu) / (n - 1)
    sigma = np.sqrt(var) + EPS
    return mu, sigma, 1.0 / sigma


# ---------------------------------------------------------------- kernel 2
def _main_body(nc):
    ap = nc.dram_tensor("ap", [I, 128, COLS], BF16, kind="ExternalInput")
    ident = nc.dram_tensor("ident", [128, 128], BF16, kind="ExternalInput")
    aux = nc.dram_tensor("aux", [128, 18], F32, kind="ExternalInput")
    outd = nc.dram_tensor("out", [128, 8], F32, kind="ExternalOutput")

    with tile.TileContext(nc) as tc:
        with (
            tc.tile_pool(name="xpool", bufs=1) as xpool,
            tc.tile_pool(name="wpool", bufs=1) as wpool,
            tc.tile_pool(name="ps", bufs=1, space="PSUM") as ps,
            tc.tile_pool(name="work", bufs=8) as work,
            tc.tile_pool(name="outp", bufs=1) as outp,
        ):
            idt = wpool.tile([128, 128], BF16, name="idt", tag="ident")
            nc.sync.dma_start(idt[:], ident[:])
            auxt = outp.tile([128, 18], F32, name="auxt", tag="aux")
            nc.scalar.dma_start(auxt[:], aux[:])

            planes = _chunked_plane_dma(nc, xpool, ap, "x", MAIN_NCHUNK)

            # 16 diagonal weights diag(N_ci/w_i) built on the idle GpSimd
            diags = {}
            for ci in range(2):
                for i in range(I):
                    k = ci * 8 + i
                    dtile = wpool.tile([128, 128], BF16, tag="diag",
                                       name=f"d{k}", bufs=16)
                    nc.gpsimd.tensor_scalar(dtile[:], idt[:],
                                            auxt[:, 2 + k:3 + k], None,
                                            op0=mybir.AluOpType.mult)
                    diags[(ci, i)] = dtile

            psum = [ps.tile([128, COLS], F32, tag=f"ps{k}", name=f"psum{k}")
                    for k in range(3)]
            out = outp.tile([128, 8], F32, name="out", tag="out")
            t_ps, r0_ps, r1_ps = psum
            e = work.tile([128, COLS], F32, name="e", tag="e")
            negaM = work.tile([128, 2], F32, name="negaM", tag="negaM")

            for i in range(I):
                t, c0 = planes[i]
                nc.tensor.matmul(psum[0][:], idt[:], t[:, c0:c0 + COLS],
                                 start=(i == 0), stop=(i == I - 1),
                                 skip_group_check=True)

            nc.vector.tensor_reduce(
                out[:, 0:2], t_ps[:].rearrange("p (b j) -> p b j", b=2),
                axis=mybir.AxisListType.X, op=mybir.AluOpType.max)
            nc.vector.tensor_scalar(negaM[:], out[:, 0:2], auxt[:, 1:2], None,
                                    op0=mybir.AluOpType.mult)
            for bh in range(2):
                sl = slice(bh * L, (bh + 1) * L)
                nc.scalar.activation(e[:, sl], t_ps[:, sl],
                                     mybir.ActivationFunctionType.Exp,
                                     bias=negaM[:, bh:bh + 1],
                                     scale=auxt[:, 0:1],
                                     accum_out=out[:, 2 + bh:3 + bh])

            for ci in range(2):
                for i in range(I):
                    t, c0 = planes[i]
                    nc.tensor.matmul(psum[1 + ci][:], diags[(ci, i)][:],
                                     t[:, c0:c0 + COLS],
                                     start=(i == 0), stop=(i == I - 1),
                                     skip_group_check=True)

            scr = [work.tile([128, COLS], F32, tag="scr", name=f"scr{k}")
                   for k in range(2)]
            for ci, rps in enumerate((r0_ps, r1_ps)):
                p_ = scr[ci]
                nc.vector.tensor_mul(p_[:], e[:], rps[:])
                nc.vector.tensor_reduce(
                    out[:, 4 + 2 * ci:6 + 2 * ci],
                    p_[:].rearrange("p (b j) -> p b j", b=2),
                    axis=mybir.AxisListType.X, op=mybir.AluOpType.add)
            nc.sync.dma_start(outd[:], out[:])
    return nc


# ---------------------------------------------------------------- host finish
def _host_finish(outs, fold, mu, sigma, alpha, y):
    O = np.stack([np.asarray(o, np.float64) for o in outs])  # [8,128,8]
    M_t = O[:, :, 0:2].transpose(0, 2, 1).reshape(-1)        # order core,bh,p
    denom_tok = O[:, :, 2:4].transpose(0, 2, 1).reshape(-1)
    G2 = np.stack([O[:, :, 4:6].transpose(0, 2, 1).reshape(-1),
                   O[:, :, 6:8].transpose(0, 2, 1).reshape(-1)], axis=1)
    a_cls, sumw, n1, v2, t2 = (fold["a_cls"], fold["sumw"], fold["n1"],
                               fold["v2"], fold["t2"])
    l_shift = alpha * M_t - alpha * mu * sumw
    m_full = np.maximum(l_shift, a_cls)
    scale_tok = np.exp(l_shift - m_full)
    e_cls = np.exp(a_cls - m_full)
    denom = denom_tok * scale_tok + e_cls
    S_cls = e_cls / denom
    gN = G2 * scale_tok[:, None] / denom[:, None]
    z = (gN - (mu * (1.0 - S_cls))[:, None] * n1[None, :]) * alpha \
        + S_cls[:, None] * v2[None, :] + t2[None, :]
    zmax = z.max(axis=1)
    lse = zmax + np.log(np.exp(z[:, 0] - zmax) + np.exp(z[:, 1] - zmax))
    y = np.asarray(y).astype(np.int64).reshape(-1)
    zy = np.take_along_axis(z, y[:, None], axis=1)[:, 0]
    return (lse - zy).mean()


# ---------------------------------------------------------------- entry point
_NC_CACHE = {}


def _get_ncs():
    if "stats" not in _NC_CACHE:
        nc = bacc.Bacc("TRN2", target_bir_lowering=False, debug=False,
                       num_devices=NCORES)
        _stats_body(nc)
        nc.compile()
        _NC_CACHE["stats"] = nc
    if "main" not in _NC_CACHE:
        nc = bacc.Bacc("TRN2", target_bir_lowering=False, debug=False,
                       num_devices=NCORES)
        _main_body(nc)
        nc.compile()
        _NC_CACHE["main"] = nc
    return _NC_CACHE["stats"], _NC_CACHE["main"]


def kernel(X, y, W1, cls_tok, W_q, W_k, W_v, W_t, W2):
    fold = _fold_weights(W1, cls_tok, W_q, W_k, W_v, W_t, W2)
    per_core = _prep_inputs(X, fold["w"])
    nc_stats, nc_main = _get_ncs()

    core_ids = list(range(NCORES))
    in1 = [{"sp": ap} for ap in per_core]
    res1 = run_bass_kernel_spmd(nc_stats, in1, core_ids=core_ids)
    mu, sigma, alpha = _host_stats(res1.results, fold["w"])

    identity = np.eye(128, dtype=bf16)
    aux = _build_aux(fold, alpha)
    in2 = [{"ap": ap, "ident": identity, "aux": aux} for ap in per_core]
    res2 = run_bass_kernel_spmd(nc_main, in2, core_ids=core_ids)
    loss = _host_finish([r["out"] for r in res2.results], fold, mu, sigma,
                        alpha, y)
    return np.float32(loss)
